# revision 6
# baseline (speedup 1.0000x reference)
"""Trainium2 Bass kernel for nn_Attention_31705448579931.

Multi-head attention (b=16, L=784, dim=384, H=8, qk=32, v=128) with a
bicubic-resampled relative-position bias:

    out = proj( softmax(q k^T/sqrt(d) + M ab M^T) v )

Sharding: data-parallel over batch — each of the 8 NeuronCores handles 2
batches and all 8 heads. The (batch-independent) bias interpolation is
replicated per core, computed head-at-a-time and fused into softmax via
exp(S+B) = exp(S) * exp(B).

Device layout highlights:
  - attention is computed k-major: S^T tiles (kpos on partitions, q on free)
    so exp(S^T) is directly the lhsT-side operand for the P@V matmul.
  - softmax denominators via an ones-vector matmul on the TensorEngine,
    reciprocal on VectorE, partition-broadcast on GpSimd, and a column-scale
    fused into the PSUM->SBUF eviction of the attention output.
  - the bias interp phase 2 exploits the 4-tap bicubic structure of the
    interpolation matrix (contraction 625 -> ~96 rows per output tile).

All matmuls run in bf16 (inputs pre-cast on host) except the two bias-interp
phases which run float32r over f32 data.
"""

import numpy as np
import ml_dtypes

import concourse.bass as bass
import concourse.mybir as mybir
import concourse.tile as tile
from concourse import bacc
from concourse.bass_utils import run_bass_kernel_spmd

N_CORES = 8
B = 16          # global batch
BC = B // N_CORES  # batches per core
L = 784
DIM = 384
H = 8
QK = 32
VD = 128
RES = 25
N = RES * RES   # 625
A_CUBIC = -0.75
SCALE = QK ** -0.5

LT = 7          # l tiles of 112
LTS = 112
NT = 5          # 625 tiles of 125
NTS = 125
F32 = mybir.dt.float32
F32R = mybir.dt.float32r
BF16 = mybir.dt.bfloat16
BF16_NP = ml_dtypes.bfloat16

NSPLITS = [(0, 512), (512, L)]  # free-dim chunks for 784-wide matmul outputs


def _cubic_weight(x):
    ax = np.abs(x)
    a = A_CUBIC
    w1 = ((a + 2.0) * ax - (a + 3.0)) * ax * ax + 1.0
    w2 = a * (((ax - 5.0) * ax + 8.0) * ax - 4.0)
    return np.where(ax <= 1.0, w1, np.where(ax < 2.0, w2, 0.0)).astype(np.float32)


def interp_matrix(Lo, Li):
    """Dense 1-D bicubic resampling matrix (Lo, Li), matches reference."""
    scale = Li / Lo
    src = (np.arange(Lo, dtype=np.float32) + 0.5) * scale - 0.5
    f = np.floor(src)
    t = (src - f).astype(np.float32)
    ws = np.stack(
        [_cubic_weight(t + 1.0), _cubic_weight(t), _cubic_weight(1.0 - t),
         _cubic_weight(2.0 - t)], axis=1)
    idx = f.astype(np.int32)[:, None] + np.arange(-1, 3, dtype=np.int32)[None, :]
    idx = np.clip(idx, 0, Li - 1)
    M = np.zeros((Lo, Li), dtype=np.float32)
    np.add.at(M, (np.arange(Lo)[:, None], idx), ws)
    return M


def _tap_windows():
    """Per l-tile row window [lo, hi) of MT rows feeding that tile (4-tap)."""
    scale = N / L
    src = (np.arange(L, dtype=np.float64) + 0.5) * scale - 0.5
    f = np.floor(src).astype(np.int64)
    lo_tap = np.clip(f - 1, 0, N - 1)
    hi_tap = np.clip(f + 2, 0, N - 1)
    wins = []
    for pt in range(LT):
        sl = slice(pt * LTS, (pt + 1) * LTS)
        wins.append((int(lo_tap[sl].min()), int(hi_tap[sl].max()) + 1))
    return wins


_BUILD_CACHE = {}


def build():
    if "nc" in _BUILD_CACHE:
        return _BUILD_CACHE["nc"]

    nc = bacc.Bacc("TRN2", target_bir_lowering=False, debug=False,
                   num_devices=N_CORES)

    xT_e = nc.dram_tensor("xT", [BC, DIM, L], BF16, kind="ExternalInput")
    wqkvT_e = nc.dram_tensor("wqkvT", [DIM, 1536], BF16, kind="ExternalInput")
    wprojT_e = nc.dram_tensor("wprojT", [H * VD, DIM], BF16, kind="ExternalInput")
    bproj_e = nc.dram_tensor("bproj", [1, DIM], F32, kind="ExternalInput")
    mt_e = nc.dram_tensor("mt", [N, L], BF16, kind="ExternalInput")
    ab_e = nc.dram_tensor("ab", [H, N, N], BF16, kind="ExternalInput")
    out_e = nc.dram_tensor("out", [BC, L, DIM], F32, kind="ExternalOutput")

    wins = _tap_windows()

    with tile.TileContext(nc) as tc:
        with (
            tc.tile_pool(name="const", bufs=1) as constp,
            tc.tile_pool(name="wq", bufs=1) as wqp,
            tc.tile_pool(name="x", bufs=1) as xp,
            tc.tile_pool(name="v", bufs=1) as vp,
            tc.tile_pool(name="qk", bufs=1) as qkp,
            tc.tile_pool(name="bias", bufs=1) as biasp,
            tc.tile_pool(name="attn", bufs=2) as attnp,
            tc.tile_pool(name="mis", bufs=1) as misp,
            tc.tile_pool(name="ot", bufs=1) as otp,
            tc.tile_pool(name="ps", bufs=2, space="PSUM") as psp,
            tc.tile_pool(name="psacc", bufs=1, space="PSUM") as psaccp,
        ):
            # ---- constants -------------------------------------------------
            bproj_row = constp.tile([1, DIM], F32, tag="bprow")
            nc.sync.dma_start(bproj_row[:], bproj_e[:, :])
            bpb = constp.tile([128, DIM], F32, tag="bpb")
            nc.gpsimd.partition_broadcast(bpb[:], bproj_row[:])

            ones_l = constp.tile([LTS, 1], BF16, tag="ones")
            nc.any.memset(ones_l[:], 1.0)

            # weights
            wq_sb = []
            for kc in range(3):
                t = wqp.tile([128, 1536], BF16, tag=f"wq{kc}")
                nc.sync.dma_start(t[:], wqkvT_e[kc * 128:(kc + 1) * 128, :])
                wq_sb.append(t)
            wproj_sb = []
            for h in range(H):
                t = wqp.tile([128, DIM], BF16, tag=f"wp{h}")
                nc.sync.dma_start(t[:], wprojT_e[h * VD:(h + 1) * VD, :])
                wproj_sb.append(t)
            # dense MT tiles (phase 1 rhs)
            mt_sb = []
            for ic in range(NT):
                t = wqp.tile([NTS, L], BF16, tag=f"mt{ic}")
                nc.sync.dma_start(t[:], mt_e[ic * NTS:(ic + 1) * NTS, :])
                mt_sb.append(t)
            # windowed MT tiles (phase 2 lhsT)
            mtwin_sb = []
            for pt in range(LT):
                lo, hi = wins[pt]
                t = wqp.tile([hi - lo, LTS], BF16, tag=f"mtw{pt}")
                nc.sync.dma_start(t[:], mt_e[lo:hi, pt * LTS:(pt + 1) * LTS])
                mtwin_sb.append(t)

            # x^T
            xT_sb = [[None] * 3 for _ in range(BC)]
            for b in range(BC):
                for kc in range(3):
                    t = xp.tile([128, L], BF16, tag=f"x{b}{kc}")
                    nc.sync.dma_start(t[:], xT_e[b, kc * 128:(kc + 1) * 128, :])
                    xT_sb[b][kc] = t

            # ---- V projection (l-major: kpos on partitions) ---------------
            v_sb = [[None] * LT for _ in range(BC)]
            for b in range(BC):
                for lt in range(LT):
                    pv = psp.tile([LTS, 1024], F32, tag="ps")
                    for half in range(2):
                        cs, ce = 512 + half * 512, 1024 + half * 512
                        for kc in range(3):
                            nc.tensor.matmul(
                                pv[:, half * 512:(half + 1) * 512],
                                lhsT=xT_sb[b][kc][:, lt * LTS:(lt + 1) * LTS],
                                rhs=wq_sb[kc][:, cs:ce],
                                start=(kc == 0), stop=(kc == 2),
                            )
                    vt = vp.tile([LTS, 1024], BF16, tag=f"v{b}{lt}")
                    nc.scalar.copy(vt[:], pv[:])
                    v_sb[b][lt] = vt

            # ---- per-head loop --------------------------------------------
            ot_sb = [[None] * H for _ in range(BC)]
            for h in range(H):
                # --- q/k for a head pair (even h computes h and h+1) -------
                if h % 2 == 0:
                    mt_i = h // 2
                    qk_tiles = []
                    for b in range(BC):
                        pqk = psp.tile([128, L], F32, tag="ps")
                        for (ns, ne) in NSPLITS:
                            for kc in range(3):
                                nc.tensor.matmul(
                                    pqk[:, ns:ne],
                                    lhsT=wq_sb[kc][:, mt_i * 128:(mt_i + 1) * 128],
                                    rhs=xT_sb[b][kc][:, ns:ne],
                                    start=(kc == 0), stop=(kc == 2),
                                )
                        for hh in range(2):
                            qt = qkp.tile([QK, L], BF16, tag=f"q{b}{hh}")
                            kt_ = qkp.tile([QK, L], BF16, tag=f"k{b}{hh}")
                            # scale q by 1/sqrt(d) here (cheaper than on S)
                            nc.scalar.mul(qt[:], pqk[hh * 64:hh * 64 + 32, :], SCALE)
                            nc.scalar.copy(kt_[:], pqk[hh * 64 + 32:hh * 64 + 64, :])
                            qk_tiles.append((qt, kt_))
                    # qk_tiles layout: [(b0,h), (b0,h+1), (b1,h), (b1,h+1)]
                    _qk_cache = qk_tiles
                q_of = lambda b: _qk_cache[b * 2 + (h % 2)][0]
                k_of = lambda b: _qk_cache[b * 2 + (h % 2)][1]

                # --- bias for head h: expB^T tiles -------------------------
                ab_sb = []
                for ic in range(NT):
                    t = biasp.tile([NTS, N], BF16, tag=f"ab{ic}")
                    nc.sync.dma_start(t[:], ab_e[h, ic * NTS:(ic + 1) * NTS, :])
                    ab_sb.append(t)
                # phase 1: U[j, o] = sum_i ab[i, j] * MT[i, o], computed
                # directly as the overlapping row-window tiles phase 2 needs
                # (window slicing happens on ab's free dim, which is
                # unrestricted; partition bases stay 0).
                uwin_sb = []
                for pt in range(LT):
                    lo, hi = wins[pt]
                    pu = psp.tile([hi - lo, L], F32, tag="ps")
                    for (ns, ne) in NSPLITS:
                        for ic in range(NT):
                            nc.tensor.matmul(
                                pu[:, ns:ne],
                                lhsT=ab_sb[ic][:, lo:hi],
                                rhs=mt_sb[ic][:, ns:ne],
                                start=(ic == 0), stop=(ic == NT - 1),
                            )
                    t = biasp.tile([hi - lo, L], BF16, tag=f"uw{pt}")
                    nc.scalar.copy(t[:], pu[:])
                    uwin_sb.append(t)
                # phase 2 (4-tap sparse): B^T tile pt, then exp -> bf16
                expb_sb = []
                for pt in range(LT):
                    lo, hi = wins[pt]
                    pb = psp.tile([LTS, L], F32, tag="ps")
                    for (ns, ne) in NSPLITS:
                        nc.tensor.matmul(
                            pb[:, ns:ne],
                            lhsT=mtwin_sb[pt][:],
                            rhs=uwin_sb[pt][:, ns:ne],
                            start=True, stop=True,
                        )
                    eb = biasp.tile([LTS, L], BF16, tag=f"eb{pt}")
                    nc.scalar.activation(eb[:], pb[:],
                                         mybir.ActivationFunctionType.Exp)
                    expb_sb.append(eb)

                # --- attention for (h, b) ----------------------------------
                for b in range(BC):
                    pt_tiles = []
                    for kt in range(LT):
                        ps_s = psp.tile([LTS, L], F32, tag="ps")
                        for (ns, ne) in NSPLITS:
                            nc.tensor.matmul(
                                ps_s[:, ns:ne],
                                lhsT=k_of(b)[:, kt * LTS:(kt + 1) * LTS],
                                rhs=q_of(b)[:, ns:ne],
                                start=True, stop=True,
                            )
                        es = attnp.tile([LTS, L], BF16, tag="expS")
                        nc.scalar.activation(es[:], ps_s[:],
                                             mybir.ActivationFunctionType.Exp)
                        ptile = attnp.tile([LTS, L], BF16, tag=f"pT{kt}")
                        nc.vector.tensor_mul(ptile[:], es[:], expb_sb[kt][:])
                        pt_tiles.append(ptile)
                    ps_o = psaccp.tile([VD, L], F32, tag="pso")
                    ps_one = psaccp.tile([1, L], F32, tag="psone")
                    for kt in range(LT):
                        for (ns, ne) in NSPLITS:
                            nc.tensor.matmul(
                                ps_o[:, ns:ne],
                                lhsT=v_sb[b][kt][:, h * VD:(h + 1) * VD],
                                rhs=pt_tiles[kt][:, ns:ne],
                                start=(kt == 0), stop=(kt == LT - 1),
                            )
                            nc.tensor.matmul(
                                ps_one[:, ns:ne],
                                lhsT=ones_l[:],
                                rhs=pt_tiles[kt][:, ns:ne],
                                start=(kt == 0), stop=(kt == LT - 1),
                            )
                    rden = misp.tile([1, L], F32, tag="rden")
                    nc.vector.reciprocal(rden[:], ps_one[:])
                    rdb = misp.tile([VD, L], F32, tag="rdb")
                    nc.gpsimd.partition_broadcast(rdb[:], rden[:])
                    ot = otp.tile([VD, L], BF16, tag=f"ot{b}{h}")
                    nc.vector.tensor_mul(ot[:], ps_o[:], rdb[:])
                    ot_sb[b][h] = ot

            # ---- output projection ----------------------------------------
            for b in range(BC):
                for lt in range(LT):
                    py = psp.tile([LTS, DIM], F32, tag="ps")
                    for h in range(H):
                        nc.tensor.matmul(
                            py[:],
                            lhsT=ot_sb[b][h][:, lt * LTS:(lt + 1) * LTS],
                            rhs=wproj_sb[h][:],
                            start=(h == 0), stop=(h == H - 1),
                        )
                    ysb = misp.tile([LTS, DIM], F32, tag="y")
                    nc.vector.tensor_add(ysb[:], py[:], bpb[:LTS, :])
                    nc.sync.dma_start(
                        out_e[b, lt * LTS:(lt + 1) * LTS, :], ysb[:])

    nc.compile()
    _BUILD_CACHE["nc"] = nc
    return nc


def _prep_in_maps(inputs):
    x = np.asarray(inputs["x"], dtype=np.float32)
    Wqkv = np.asarray(inputs["Wqkv"], dtype=np.float32)
    Wproj = np.asarray(inputs["Wproj"], dtype=np.float32)
    bproj = np.asarray(inputs["bproj"], dtype=np.float32)
    ab_table = np.asarray(inputs["ab_table"], dtype=np.float32)
    bias_idxs = np.asarray(inputs["bias_idxs"])

    # reorder qkv weight rows: [q0 k0 q1 k1 ... q7 k7 | v0 v1 ... v7]
    w3 = Wqkv.reshape(H, 2 * QK + VD, DIM)
    order = []
    for h in range(H):
        pass
    qk_rows = np.concatenate(
        [np.concatenate([w3[h, :QK], w3[h, QK:2 * QK]], axis=0) for h in range(H)],
        axis=0)                     # (512, 384)
    v_rows = np.concatenate([w3[h, 2 * QK:] for h in range(H)], axis=0)  # (1024,384)
    wqkvT = np.ascontiguousarray(
        np.concatenate([qk_rows, v_rows], axis=0).T).astype(BF16_NP)  # (384,1536)

    wprojT = np.ascontiguousarray(Wproj.T).astype(BF16_NP)  # (1024, 384)
    mt = np.ascontiguousarray(interp_matrix(L, N).T).astype(BF16_NP)  # (625, 784)
    ab = np.ascontiguousarray(ab_table[:, bias_idxs]).astype(BF16_NP)
    bproj2 = np.ascontiguousarray(bproj.reshape(1, DIM))

    in_maps = []
    for c in range(N_CORES):
        xT = np.ascontiguousarray(
            x[c * BC:(c + 1) * BC].transpose(0, 2, 1)).astype(BF16_NP)
        in_maps.append({
            "xT": xT,
            "wqkvT": wqkvT,
            "wprojT": wprojT,
            "bproj": bproj2,
            "mt": mt,
            "ab": ab,
        })
    return in_maps


def _run(inputs, trace=False, **kw):
    nc = build()
    in_maps = _prep_in_maps(inputs)
    res = run_bass_kernel_spmd(nc, in_maps, core_ids=list(range(N_CORES)),
                               trace=trace, **kw)
    out = np.concatenate([res.results[c]["out"] for c in range(N_CORES)], axis=0)
    return out, res


def kernel(**inputs) -> np.ndarray:
    out, _ = _run(inputs, trace=False)
    return out


# revision 8
# speedup vs baseline: 1.0495x; 1.0495x over previous
"""Trainium2 Bass kernel for nn_Attention_31705448579931.

Multi-head attention (b=16, L=784, dim=384, H=8, qk=32, v=128) with a
bicubic-resampled relative-position bias:

    out = proj( softmax(q k^T/sqrt(d) + M ab M^T) v )

Sharding: data-parallel over batch — each of the 8 NeuronCores handles 2
batches and all 8 heads. The (batch-independent) bias interpolation is
replicated per core, computed head-at-a-time and fused into softmax via
exp(S+B) = exp(S) * exp(B).

Device layout highlights:
  - attention is computed k-major: S^T tiles (kpos on partitions, q on free)
    so exp(S^T) is directly the lhsT-side operand for the P@V matmul.
  - softmax denominators via an ones-vector matmul on the TensorEngine,
    reciprocal on VectorE, partition-broadcast on GpSimd, and a column-scale
    fused into the PSUM->SBUF eviction of the attention output.
  - the bias interp phase 2 exploits the 4-tap bicubic structure of the
    interpolation matrix (contraction 625 -> ~96 rows per output tile).

All matmuls run in bf16 (inputs pre-cast on host) except the two bias-interp
phases which run float32r over f32 data.
"""

import numpy as np
import ml_dtypes

import concourse.bass as bass
import concourse.mybir as mybir
import concourse.tile as tile
from concourse import bacc
from concourse.bass_utils import run_bass_kernel_spmd

N_CORES = 8
B = 16          # global batch
BC = B // N_CORES  # batches per core
L = 784
DIM = 384
H = 8
QK = 32
VD = 128
RES = 25
N = RES * RES   # 625
A_CUBIC = -0.75
SCALE = QK ** -0.5

LT = 7          # l tiles of 112
LTS = 112
NT = 5          # 625 tiles of 125
NTS = 125
F32 = mybir.dt.float32
F32R = mybir.dt.float32r
BF16 = mybir.dt.bfloat16
BF16_NP = ml_dtypes.bfloat16

NSPLITS = [(0, 512), (512, L)]  # free-dim chunks for 784-wide matmul outputs


def _cubic_weight(x):
    ax = np.abs(x)
    a = A_CUBIC
    w1 = ((a + 2.0) * ax - (a + 3.0)) * ax * ax + 1.0
    w2 = a * (((ax - 5.0) * ax + 8.0) * ax - 4.0)
    return np.where(ax <= 1.0, w1, np.where(ax < 2.0, w2, 0.0)).astype(np.float32)


def interp_matrix(Lo, Li):
    """Dense 1-D bicubic resampling matrix (Lo, Li), matches reference."""
    scale = Li / Lo
    src = (np.arange(Lo, dtype=np.float32) + 0.5) * scale - 0.5
    f = np.floor(src)
    t = (src - f).astype(np.float32)
    ws = np.stack(
        [_cubic_weight(t + 1.0), _cubic_weight(t), _cubic_weight(1.0 - t),
         _cubic_weight(2.0 - t)], axis=1)
    idx = f.astype(np.int32)[:, None] + np.arange(-1, 3, dtype=np.int32)[None, :]
    idx = np.clip(idx, 0, Li - 1)
    M = np.zeros((Lo, Li), dtype=np.float32)
    np.add.at(M, (np.arange(Lo)[:, None], idx), ws)
    return M


def _tap_windows():
    """Per l-tile row window [lo, hi) of MT rows feeding that tile (4-tap)."""
    scale = N / L
    src = (np.arange(L, dtype=np.float64) + 0.5) * scale - 0.5
    f = np.floor(src).astype(np.int64)
    lo_tap = np.clip(f - 1, 0, N - 1)
    hi_tap = np.clip(f + 2, 0, N - 1)
    wins = []
    for pt in range(LT):
        sl = slice(pt * LTS, (pt + 1) * LTS)
        wins.append((int(lo_tap[sl].min()), int(hi_tap[sl].max()) + 1))
    return wins


_BUILD_CACHE = {}


def build():
    if "nc" in _BUILD_CACHE:
        return _BUILD_CACHE["nc"]

    nc = bacc.Bacc("TRN2", target_bir_lowering=False, debug=False,
                   num_devices=N_CORES)

    xT_e = nc.dram_tensor("xT", [BC, DIM, L], BF16, kind="ExternalInput")
    wqkvT_e = nc.dram_tensor("wqkvT", [DIM, 1536], BF16, kind="ExternalInput")
    wprojT_e = nc.dram_tensor("wprojT", [H * VD, DIM], BF16, kind="ExternalInput")
    bproj_e = nc.dram_tensor("bproj", [1, DIM], F32, kind="ExternalInput")
    mt_e = nc.dram_tensor("mt", [N, L], BF16, kind="ExternalInput")
    ab_e = nc.dram_tensor("ab", [H, N, N], BF16, kind="ExternalInput")
    out_e = nc.dram_tensor("out", [BC, L, DIM], F32, kind="ExternalOutput")

    wins = _tap_windows()

    with tile.TileContext(nc) as tc:
        with (
            tc.tile_pool(name="const", bufs=1) as constp,
            tc.tile_pool(name="wq", bufs=1) as wqp,
            tc.tile_pool(name="x", bufs=1) as xp,
            tc.tile_pool(name="v", bufs=1) as vp,
            tc.tile_pool(name="qk", bufs=1) as qkp,
            tc.tile_pool(name="bias", bufs=1) as biasp,
            tc.tile_pool(name="attn", bufs=2) as attnp,
            tc.tile_pool(name="mis", bufs=1) as misp,
            tc.tile_pool(name="ot", bufs=1) as otp,
            tc.tile_pool(name="ps", bufs=2, space="PSUM") as psp,
            tc.tile_pool(name="psacc", bufs=1, space="PSUM") as psaccp,
        ):
            # ---- constants -------------------------------------------------
            bproj_row = constp.tile([1, DIM], F32, tag="bprow")
            nc.sync.dma_start(bproj_row[:], bproj_e[:, :])
            bpb = constp.tile([128, DIM], F32, tag="bpb")
            nc.gpsimd.partition_broadcast(bpb[:], bproj_row[:])

            ones_l = constp.tile([LTS, 1], BF16, tag="ones")
            nc.any.memset(ones_l[:], 1.0)

            # weights
            wq_sb = []
            for kc in range(3):
                t = wqp.tile([128, 1536], BF16, tag=f"wq{kc}")
                nc.sync.dma_start(t[:], wqkvT_e[kc * 128:(kc + 1) * 128, :])
                wq_sb.append(t)
            wproj_sb = []
            for h in range(H):
                t = wqp.tile([128, DIM], BF16, tag=f"wp{h}")
                nc.sync.dma_start(t[:], wprojT_e[h * VD:(h + 1) * VD, :])
                wproj_sb.append(t)
            # dense MT tiles (phase 1 rhs)
            mt_sb = []
            for ic in range(NT):
                t = wqp.tile([NTS, L], BF16, tag=f"mt{ic}")
                nc.sync.dma_start(t[:], mt_e[ic * NTS:(ic + 1) * NTS, :])
                mt_sb.append(t)
            # windowed MT tiles (phase 2 lhsT)
            mtwin_sb = []
            for pt in range(LT):
                lo, hi = wins[pt]
                t = wqp.tile([hi - lo, LTS], BF16, tag=f"mtw{pt}")
                nc.sync.dma_start(t[:], mt_e[lo:hi, pt * LTS:(pt + 1) * LTS])
                mtwin_sb.append(t)

            # x^T
            xT_sb = [[None] * 3 for _ in range(BC)]
            for b in range(BC):
                for kc in range(3):
                    t = xp.tile([128, L], BF16, tag=f"x{b}{kc}")
                    nc.sync.dma_start(t[:], xT_e[b, kc * 128:(kc + 1) * 128, :])
                    xT_sb[b][kc] = t

            # ---- V projection (l-major: kpos on partitions) ---------------
            v_sb = [[None] * LT for _ in range(BC)]
            for b in range(BC):
                for lt in range(LT):
                    pv = psp.tile([LTS, 1024], F32, tag="ps")
                    for half in range(2):
                        for kc in range(3):
                            nc.tensor.matmul(
                                pv[:, half * 512:(half + 1) * 512],
                                lhsT=xT_sb[b][kc][:, lt * LTS:(lt + 1) * LTS],
                                rhs=wq_sb[kc][:, 512 + half * 512:1024 + half * 512],
                                start=(kc == 0), stop=(kc == 2),
                            )
                    vt = vp.tile([LTS, 1024], BF16, tag=f"v{b}{lt}")
                    nc.vector.tensor_copy(vt[:], pv[:])
                    v_sb[b][lt] = vt

            # ---- per-head loop --------------------------------------------
            ot_sb = [[None] * H for _ in range(BC)]
            for h in range(H):
                # --- q/k for a head pair (even h computes h and h+1) -------
                if h % 2 == 0:
                    mt_i = h // 2
                    qk_tiles = []
                    for b in range(BC):
                        pqk = psp.tile([128, L], F32, tag="ps")
                        for (ns, ne) in NSPLITS:
                            for kc in range(3):
                                nc.tensor.matmul(
                                    pqk[:, ns:ne],
                                    lhsT=wq_sb[kc][:, mt_i * 128:(mt_i + 1) * 128],
                                    rhs=xT_sb[b][kc][:, ns:ne],
                                    start=(kc == 0), stop=(kc == 2),
                                )
                        for hh in range(2):
                            qt = qkp.tile([QK, L], BF16, tag=f"q{b}{hh}")
                            kt_ = qkp.tile([QK, L], BF16, tag=f"k{b}{hh}")
                            # q pre-scaled by 1/sqrt(d) on host (folded into Wq)
                            nc.vector.tensor_copy(qt[:], pqk[hh * 64:hh * 64 + 32, :])
                            nc.vector.tensor_copy(kt_[:], pqk[hh * 64 + 32:hh * 64 + 64, :])
                            qk_tiles.append((qt, kt_))
                    # qk_tiles layout: [(b0,h), (b0,h+1), (b1,h), (b1,h+1)]
                    _qk_cache = qk_tiles
                q_of = lambda b: _qk_cache[b * 2 + (h % 2)][0]
                k_of = lambda b: _qk_cache[b * 2 + (h % 2)][1]

                # --- bias for head h: expB^T tiles -------------------------
                ab_sb = []
                for ic in range(NT):
                    t = biasp.tile([NTS, N], BF16, tag=f"ab{ic}")
                    nc.sync.dma_start(t[:], ab_e[h, ic * NTS:(ic + 1) * NTS, :])
                    ab_sb.append(t)
                # phase 1: U[j, o] = sum_i ab[i, j] * MT[i, o], computed
                # directly as the overlapping row-window tiles phase 2 needs
                # (window slicing happens on ab's free dim, which is
                # unrestricted; partition bases stay 0).
                uwin_sb = []
                for pt in range(LT):
                    lo, hi = wins[pt]
                    pu = psp.tile([hi - lo, L], F32, tag="ps")
                    for (ns, ne) in NSPLITS:
                        for ic in range(NT):
                            nc.tensor.matmul(
                                pu[:, ns:ne],
                                lhsT=ab_sb[ic][:, lo:hi],
                                rhs=mt_sb[ic][:, ns:ne],
                                start=(ic == 0), stop=(ic == NT - 1),
                            )
                    t = biasp.tile([hi - lo, L], BF16, tag=f"uw{pt}")
                    nc.vector.tensor_copy(t[:], pu[:])
                    uwin_sb.append(t)
                # phase 2 (4-tap sparse): B^T tile pt, then exp -> bf16
                expb_sb = []
                for pt in range(LT):
                    lo, hi = wins[pt]
                    pb = psp.tile([LTS, L], F32, tag="ps")
                    for (ns, ne) in NSPLITS:
                        nc.tensor.matmul(
                            pb[:, ns:ne],
                            lhsT=mtwin_sb[pt][:],
                            rhs=uwin_sb[pt][:, ns:ne],
                            start=True, stop=True,
                        )
                    eb = biasp.tile([LTS, L], BF16, tag=f"eb{pt}")
                    nc.scalar.activation(eb[:], pb[:],
                                         mybir.ActivationFunctionType.Exp)
                    expb_sb.append(eb)

                # --- attention for (h, b) ----------------------------------
                for b in range(BC):
                    pt_tiles = []
                    for kt in range(LT):
                        ps_s = psp.tile([LTS, L], F32, tag="ps")
                        for (ns, ne) in NSPLITS:
                            nc.tensor.matmul(
                                ps_s[:, ns:ne],
                                lhsT=k_of(b)[:, kt * LTS:(kt + 1) * LTS],
                                rhs=q_of(b)[:, ns:ne],
                                start=True, stop=True,
                            )
                        es = attnp.tile([LTS, L], BF16, tag="expS")
                        nc.scalar.activation(es[:], ps_s[:],
                                             mybir.ActivationFunctionType.Exp)
                        ptile = attnp.tile([LTS, L], BF16, tag=f"pT{kt}")
                        nc.vector.tensor_mul(ptile[:], es[:], expb_sb[kt][:])
                        pt_tiles.append(ptile)
                    ps_o = psaccp.tile([VD, L], F32, tag="pso")
                    ps_one = psaccp.tile([1, L], F32, tag="psone")
                    for kt in range(LT):
                        for (ns, ne) in NSPLITS:
                            nc.tensor.matmul(
                                ps_o[:, ns:ne],
                                lhsT=v_sb[b][kt][:, h * VD:(h + 1) * VD],
                                rhs=pt_tiles[kt][:, ns:ne],
                                start=(kt == 0), stop=(kt == LT - 1),
                            )
                            nc.tensor.matmul(
                                ps_one[:, ns:ne],
                                lhsT=ones_l[:],
                                rhs=pt_tiles[kt][:, ns:ne],
                                start=(kt == 0), stop=(kt == LT - 1),
                            )
                    rden = misp.tile([1, L], F32, tag="rden")
                    nc.vector.reciprocal_approx_fast(rden[:], ps_one[:])
                    rdb = misp.tile([VD, L], F32, tag="rdb")
                    nc.gpsimd.partition_broadcast(rdb[:], rden[:])
                    ot = otp.tile([VD, L], BF16, tag=f"ot{b}{h}")
                    nc.vector.tensor_mul(ot[:], ps_o[:], rdb[:])
                    ot_sb[b][h] = ot

            # ---- output projection ----------------------------------------
            for b in range(BC):
                for lt in range(LT):
                    py = psp.tile([LTS, DIM], F32, tag="ps")
                    for h in range(H):
                        nc.tensor.matmul(
                            py[:],
                            lhsT=ot_sb[b][h][:, lt * LTS:(lt + 1) * LTS],
                            rhs=wproj_sb[h][:],
                            start=(h == 0), stop=(h == H - 1),
                        )
                    ysb = misp.tile([LTS, DIM], F32, tag="y")
                    nc.vector.tensor_add(ysb[:], py[:], bpb[:LTS, :])
                    nc.sync.dma_start(
                        out_e[b, lt * LTS:(lt + 1) * LTS, :], ysb[:])

    nc.compile()
    _BUILD_CACHE["nc"] = nc
    return nc


def _prep_in_maps(inputs):
    x = np.asarray(inputs["x"], dtype=np.float32)
    Wqkv = np.asarray(inputs["Wqkv"], dtype=np.float32)
    Wproj = np.asarray(inputs["Wproj"], dtype=np.float32)
    bproj = np.asarray(inputs["bproj"], dtype=np.float32)
    ab_table = np.asarray(inputs["ab_table"], dtype=np.float32)
    bias_idxs = np.asarray(inputs["bias_idxs"])

    # reorder qkv weight rows: [q0 k0 q1 k1 ... q7 k7 | v0 v1 ... v7]
    w3 = Wqkv.reshape(H, 2 * QK + VD, DIM)
    order = []
    for h in range(H):
        pass
    qk_rows = np.concatenate(
        [np.concatenate([w3[h, :QK] * SCALE, w3[h, QK:2 * QK]], axis=0)
         for h in range(H)],
        axis=0)                     # (512, 384)
    v_rows = np.concatenate([w3[h, 2 * QK:] for h in range(H)], axis=0)  # (1024,384)
    wqkvT = np.ascontiguousarray(
        np.concatenate([qk_rows, v_rows], axis=0).T).astype(BF16_NP)  # (384,1536)

    wprojT = np.ascontiguousarray(Wproj.T).astype(BF16_NP)  # (1024, 384)
    mt = np.ascontiguousarray(interp_matrix(L, N).T).astype(BF16_NP)  # (625, 784)
    ab = np.ascontiguousarray(ab_table[:, bias_idxs]).astype(BF16_NP)
    bproj2 = np.ascontiguousarray(bproj.reshape(1, DIM))

    in_maps = []
    for c in range(N_CORES):
        xT = np.ascontiguousarray(
            x[c * BC:(c + 1) * BC].transpose(0, 2, 1)).astype(BF16_NP)
        in_maps.append({
            "xT": xT,
            "wqkvT": wqkvT,
            "wprojT": wprojT,
            "bproj": bproj2,
            "mt": mt,
            "ab": ab,
        })
    return in_maps


def _run(inputs, trace=False, **kw):
    nc = build()
    in_maps = _prep_in_maps(inputs)
    res = run_bass_kernel_spmd(nc, in_maps, core_ids=list(range(N_CORES)),
                               trace=trace, **kw)
    out = np.concatenate([res.results[c]["out"] for c in range(N_CORES)], axis=0)
    return out, res


def kernel(**inputs) -> np.ndarray:
    out, _ = _run(inputs, trace=False)
    return out
